# revision 13
# baseline (speedup 1.0000x reference)
"""Trainium2 Bass kernel for nn_CnnBasedRnn (2-layer conv-RNN).

Math: each layer computes h_t = tanh(conv3x3_stride(2,1)(concat(x_t, h_{t-1})) + b).
Because the conv input is [x_t (rows 0..63); h_{t-1} (rows 64..127)] with row
stride 2, output row i taps concat rows 2i-1..2i+1:
  rows 0..31  <- x_t only                        (bulk pass)
  row  i>=32  <- h_{t-1} rows 2i-65..2i-63       (cascade regions)
Region cascade: rows 32..47 need prev-step rows <=31, 48..55 need <=47,
56..59 need <=55, 60..61 need <=59, 62 needs <=61 -- all earlier passes.
Only row 63 self-recurses (taps prev row 63); solved by fixed-point sweeps
over the whole sequence: H <- tanh(dv + W[2] (x) shift_t(H)), contracting by
~sum|W[2,:]| per sweep.

v2 layout (slot-major, fp16): SBUF partitions = (img(2) x j(64)); S[l] is
[128, 65 slots, 257 groups]: slot 0 group g = layer_input_g[row 63], slot
1+r group g = h_{g-1}[row r].  All DMAs are then contiguous runs of 256
fp16 along the group dim (512B descriptors).  Matmuls run in fp16 (1
PE cycle/row vs 4 for fp32), accumulating in fp32 PSUM.  Output rows
0..31 / 32..62 / 63 are DMA'd out incrementally so only the tiny row-63
transfer sits after the last compute.  The layer-2 bulk pass (rows 0..30,
which don't tap layer-1 row 63) is interleaved into iterate(0)'s serial
matmul->act latency bubbles.  The per-sweep dv constant is preloaded into
PSUM by the vector engine so each sweep is one matmul + one activation.
"""

import os
import numpy as np

B, L, D, NCORES = 16, 256, 64, 8
BS = B // NCORES          # images per core


def _band(w3):
    """[64,64] banded matrix M[jin, jout] = w3[jin-jout+1] for |jin-jout|<=1."""
    M = np.zeros((D, D), np.float32)
    for dj in range(3):
        jout = np.arange(D)
        jin = jout + dj - 1
        m = (jin >= 0) & (jin < D)
        M[jin[m], jout[m]] = w3[dj]
    return M


def _bands_tensor(Wn):
    """[128, 7, 128] fp16: (l, di) -> block-diag band; slot 6 = identity."""
    out = np.zeros((128, 7, 128), np.float32)
    for l in range(2):
        for di in range(3):
            M = _band(Wn[l, di])
            out[0:64, l * 3 + di, 0:64] = M
            out[64:128, l * 3 + di, 64:128] = M
    out[:, 6, :] = np.eye(128, dtype=np.float32)
    return np.ascontiguousarray(out.astype(np.float16))


def _conv1d3(v, w3):
    out = (w3[1] * v).copy()
    out[..., :-1] += w3[2] * v[..., 1:]
    out[..., 1:] += w3[0] * v[..., :-1]
    return out


def _numpy_layer(xl, Wl, bl, n_iter):
    """Reference decomposition (for sweep-count estimation). xl: (b,L,D,D)."""
    nb = xl.shape[0]
    h = np.zeros((nb, L, D, D), np.float32)
    xpad = np.zeros((nb, L, D + 2, D), np.float32)
    xpad[:, :, 1:D + 1] = xl
    for i in range(32):
        acc = np.zeros((nb, L, D), np.float32)
        for di in range(3):
            acc = acc + _conv1d3(xpad[:, :, 2 * i + di], Wl[di])
        h[:, :, i] = np.tanh(acc + bl)

    def S_prev(slot):
        out = np.zeros((nb, L, D), np.float32)
        if slot == 0:
            out[:, :] = xl[:, :, 63]
        else:
            out[:, 1:] = h[:, :-1, slot - 1]
        return out

    for lo, hi in ((32, 47), (48, 55), (56, 59), (60, 61), (62, 62)):
        for i in range(lo, hi + 1):
            acc = np.zeros((nb, L, D), np.float32)
            for di in range(3):
                acc = acc + _conv1d3(S_prev(2 * i - 64 + di), Wl[di])
            h[:, :, i] = np.tanh(acc + bl)

    dv = bl + _conv1d3(S_prev(62), Wl[0]) + _conv1d3(S_prev(63), Wl[1])
    H = np.zeros((nb, L, D), np.float32)
    deltas = []
    for _ in range(n_iter):
        Hp = np.zeros((nb, L, D), np.float32)
        Hp[:, 1:] = H[:, :-1]
        Hn = np.tanh(dv + _conv1d3(Hp, Wl[2]))
        deltas.append(float(np.abs(Hn - H).max()))
        H = Hn
    h[:, :, 63] = H
    return h, deltas


def _estimate_sweeps(x, Wn, bn, tol=1e-3):
    """Run the decomposition on one image, count sweeps until delta < tol.

    Harness gate is 2e-2 relative; fp16 storage floors the useful delta at
    ~2e-4, so iterating further is pure latency."""
    xs = x[:1].astype(np.float32)
    nits = []
    for l in range(2):
        xs_out, deltas = _numpy_layer(xs, Wn[l], bn[l], 30)
        nit = 30
        for k, d in enumerate(deltas):
            if d < tol:
                nit = k + 1
                break
        nits.append(min(30, max(5, nit + 2)))
        xs = xs_out
    return nits


def _build_bass(bn, nits):
    import concourse.bass as bass  # noqa: F401
    import concourse.bacc as bacc
    import concourse.mybir as mybir
    import concourse.tile as tile

    f16 = mybir.dt.float16
    f32 = mybir.dt.float32
    Tanh = mybir.ActivationFunctionType.Tanh

    nc = bacc.Bacc("TRN2", target_bir_lowering=False)
    # [part, t-quarter, row, t-in-quarter]: each quarter-DMA has a contiguous
    # source, quarters alternate between the two hwdge queues (sync/scalar)
    # so the first bulk chunk can start after ~1/4 of the input has landed.
    xT = nc.dram_tensor("xT", [128, 4, D, 64], f16, kind="ExternalInput")
    bands = nc.dram_tensor("bands", [128, 7, 128], f16, kind="ExternalInput")
    outT = nc.dram_tensor("outT", [128, D, L], f16, kind="ExternalOutput")

    with tile.TileContext(nc) as tc:
        with (
            tc.tile_pool(name="persist", bufs=1) as persist,
            tc.tile_pool(name="apool", bufs=3, space="PSUM") as apool,
            tc.tile_pool(name="ipool", bufs=2, space="PSUM") as ipool,
        ):
            bsb = persist.tile([128, 7, 128], f16)
            nc.sync.dma_start(out=bsb, in_=bands[:])

            def BD(l, di):
                return bsb[:, l * 3 + di, :]

            IDN = bsb[:, 6, :]

            S = [persist.tile([128, 65, L + 1], f16, name=f"S{i}")
                 for i in range(2)]
            dvs = [persist.tile([128, L], f16, name=f"dv{i}")
                   for i in range(2)]
            bias_t = [persist.tile([128, 1], f32, name=f"bias{i}")
                      for i in range(2)]
            for i in range(2):
                nc.vector.memset(bias_t[i][:, :], float(bn[i]))
                nc.vector.memset(S[i][:, 1:65, 0:1], 0.0)
                # slot 64 is read across all groups by the first fixed-point
                # sweep (H^0 = 0); stale SBUF there would poison row 63.
                nc.vector.memset(S[i][:, 64, :], 0.0)

            xt = persist.tile([128, D, L], f16)
            for c in range(4):
                nc.sync.dma_start(out=xt[:, :, c * 64:(c + 1) * 64],
                                  in_=xT[:, c])
                # slot0[g] = x_g[row 63]
                nc.vector.tensor_copy(
                    S[0][:, 0, c * 64:(c + 1) * 64],
                    xt[:, 63, c * 64:(c + 1) * 64])

            # Spin the PE on dummy matmuls while the input DMA streams in:
            # the PE pstate ramps with continuous execution, so the first
            # real chunks would otherwise run ~1.6x slow.
            for _ in range(16):
                pw = ipool.tile([128, 256], f32, name="pw", tag="it")
                nc.tensor.matmul(pw, bsb[:, 0, :], bsb[:, 0:2, :],
                                 start=True, stop=True)

            def bulk_chunk(l, ts):
                """Rows 0..31 (l=0) / 0..30 (l=1) for 32 timesteps at ts.

                PSUM is [128, 2, 32, 16]: each 16-t half fills one aligned
                2KB bank (matmuls may not cross banks), while one activation
                drains both banks through a 4D split-group output AP."""
                nrows = 32 if l == 0 else 31
                pa = apool.tile([128, 2, 32, 16], f32, name="pa", tag="acc")
                for q in range(2):
                    tq = ts + q * 16
                    if l == 0:
                        r1 = xt[:, 0:63:2, tq:tq + 16]
                        r2 = xt[:, 1:64:2, tq:tq + 16]
                        r0 = xt[:, 1:62:2, tq:tq + 16]
                    else:
                        g = tq + 1
                        r1 = S[0][:, 1:62:2, g:g + 16]
                        r2 = S[0][:, 2:63:2, g:g + 16]
                        r0 = S[0][:, 2:61:2, g:g + 16]
                    nc.tensor.matmul(pa[:, q, 0:nrows, :], BD(l, 1), r1,
                                     start=True, stop=False)
                    nc.tensor.matmul(pa[:, q, 0:nrows, :], BD(l, 2), r2,
                                     start=False, stop=False)
                    nc.tensor.matmul(pa[:, q, 1:nrows, :], BD(l, 0), r0,
                                     start=False, stop=True)
                out = S[l][:, 1:1 + nrows, ts + 1:ts + 33].rearrange(
                    "p r (q t) -> p q r t", q=2)
                nc.scalar.activation(out, pa[:, :, 0:nrows, :], Tanh,
                                     bias=bias_t[l][:, :])

            def region_pass(l):
                """Cascade rows 32..62, region-major over the full sequence.
                Adjacent NT-tiles are paired into a 2-bank PSUM tile so one
                activation drains both."""
                Sl = S[l]
                for (ilo, ihi, NT) in ((32, 47, 32), (48, 55, 64),
                                       (56, 59, 128)):
                    n = ihi - ilo + 1
                    for t0 in range(0, L, 2 * NT):
                        pr = apool.tile([128, 2, n, NT], f32, name="pr",
                                        tag="acc")
                        for q in range(2):
                            tq = t0 + q * NT
                            for di in range(3):
                                s0 = 2 * ilo - 64 + di
                                rhs = Sl[:, s0:s0 + 2 * n - 1:2, tq:tq + NT]
                                nc.tensor.matmul(pr[:, q], BD(l, di), rhs,
                                                 start=(di == 0),
                                                 stop=(di == 2))
                        out = Sl[:, 1 + ilo:2 + ihi,
                                 t0 + 1:t0 + 2 * NT + 1].rearrange(
                                     "p r (q t) -> p q r t", q=2)
                        nc.scalar.activation(out, pr[:, :, :, :], Tanh,
                                             bias=bias_t[l][:, :])
                for (ilo, ihi) in ((60, 61), (62, 62)):
                    n = ihi - ilo + 1
                    pr = apool.tile([128, n, L], f32, name="prl", tag="acc")
                    for di in range(3):
                        s0 = 2 * ilo - 64 + di
                        rhs = Sl[:, s0:s0 + 2 * n - 1:2, 0:L]
                        nc.tensor.matmul(pr[:, :, :], BD(l, di), rhs,
                                         start=(di == 0), stop=(di == 2))
                    nc.scalar.activation(
                        Sl[:, 1 + ilo:2 + ihi, 1:L + 1],
                        pr[:, :, :], Tanh, bias=bias_t[l][:, :])

            def dv_prep(l):
                pd = ipool.tile([128, L], f32, name="pd", tag="it")
                nc.tensor.matmul(pd, BD(l, 0), S[l][:, 62, 0:L],
                                 start=True, stop=False)
                nc.tensor.matmul(pd, BD(l, 1), S[l][:, 63, 0:L],
                                 start=False, stop=True)
                nc.vector.tensor_copy(dvs[l][:, :], pd)

            def sweep(l):
                pi = ipool.tile([128, L], f32, name="pi", tag="it")
                nc.vector.tensor_copy(pi, dvs[l][:, :])
                nc.tensor.matmul(pi, BD(l, 2), S[l][:, 64, 0:L],
                                 start=False, stop=True,
                                 skip_group_check=True)
                nc.scalar.activation(S[l][:, 64, 1:L + 1], pi, Tanh,
                                     bias=bias_t[l][:, :])

            # ---- layer 1 ----
            for ts in range(0, L, 32):
                bulk_chunk(0, ts)
            region_pass(0)
            dv_prep(0)
            # iterate(0), with layer-2 bulk chunks filling the PE bubbles
            nchunks = L // 32
            for k in range(max(nits[0], nchunks)):
                if k < nits[0]:
                    sweep(0)
                if k < nchunks:
                    bulk_chunk(1, k * 32)
            # layer-2 row 31 (taps layer-1 rows 61,62,63 = slots 62,63,64)
            pr31 = apool.tile([128, L], f32, name="pr31", tag="acc")
            for di in range(3):
                nc.tensor.matmul(pr31, BD(1, di), S[0][:, 62 + di, 1:L + 1],
                                 start=(di == 0), stop=(di == 2))
            nc.scalar.activation(S[1][:, 32, 1:L + 1], pr31, Tanh,
                                 bias=bias_t[1][:, :])
            # layer-2 slot0[g] = h1_g[row 63]
            nc.vector.tensor_copy(S[1][:, 0, 0:L], S[0][:, 64, 1:L + 1])

            # rows 0..31 final -> overlap region_pass(1) with their DMA
            nc.sync.dma_start(out=outT[:, 0:32, :],
                              in_=S[1][:, 1:33, 1:L + 1])
            region_pass(1)
            # rows 32..62 final -> overlap iterate(1) with their DMA
            nc.sync.dma_start(out=outT[:, 32:63, :],
                              in_=S[1][:, 33:64, 1:L + 1])
            dv_prep(1)
            for k in range(nits[1]):
                sweep(1)
            nc.sync.dma_start(out=outT[:, 63, :], in_=S[1][:, 64, 1:L + 1])

    nc.compile()
    return nc


def kernel(x, W, b):
    import sys
    if "/opt/trn_rl_repo" not in sys.path:
        sys.path.insert(0, "/opt/trn_rl_repo")
    from concourse.bass_utils import run_bass_kernel_spmd

    x = np.ascontiguousarray(np.asarray(x, np.float32))
    Wn = np.asarray(W, np.float32)[:, 0, 0]      # (2, 3, 3)
    bn = np.asarray(b, np.float32)               # (2,)

    nits = _estimate_sweeps(x, Wn, bn)
    nc = _build_bass(bn, nits)

    bands_np = _bands_tensor(Wn)
    in_maps = []
    for c in range(NCORES):
        xc = x[c * BS:(c + 1) * BS]                      # (2, L, D, D)
        # (img, t, row, j) -> (img*j, row, t) -> [128, 2, 64, 128]
        xTc = xc.transpose(0, 3, 2, 1).reshape(128, D, L)
        xTc = np.ascontiguousarray(
            xTc.reshape(128, D, 4, 64).transpose(0, 2, 1, 3)
        ).astype(np.float16)
        in_maps.append({"xT": xTc, "bands": bands_np})

    res = run_bass_kernel_spmd(
        nc, in_maps, core_ids=list(range(NCORES)),
        trace=bool(int(os.environ.get("BASS_KERNEL_TRACE", "0"))))
    if os.environ.get("BASS_KERNEL_RESULT_PATH"):
        import pickle
        with open(os.environ["BASS_KERNEL_RESULT_PATH"], "wb") as f:
            pickle.dump({
                "exec_time_ns": res.exec_time_ns,
                "mean_exec_time_ns": res.mean_exec_time_ns,
                "trace": (res.instructions_and_trace or (None, None))[1],
                "profile_json": res.profile_json,
            }, f)

    out = np.empty((B, L, D, D), np.float32)
    for c in range(NCORES):
        r = res.results[c]
        main = r["outT"].reshape(BS, D, D, L)            # (img, j, row, t)
        out[c * BS:(c + 1) * BS] = main.transpose(0, 3, 2, 1).astype(np.float32)
    return out


# revision 14
# speedup vs baseline: 1.3279x; 1.3279x over previous
"""Trainium2 Bass kernel for nn_CnnBasedRnn (2-layer conv-RNN).

Math: each layer computes h_t = tanh(conv3x3_stride(2,1)(concat(x_t, h_{t-1})) + b).
Because the conv input is [x_t (rows 0..63); h_{t-1} (rows 64..127)] with row
stride 2, output row i taps concat rows 2i-1..2i+1:
  rows 0..31  <- x_t only                        (bulk pass)
  row  i>=32  <- h_{t-1} rows 2i-65..2i-63       (cascade regions)
Region cascade: rows 32..47 need prev-step rows <=31, 48..55 need <=47,
56..59 need <=55, 60..61 need <=59, 62 needs <=61 -- all earlier passes.
Only row 63 self-recurses (taps prev row 63); solved by fixed-point sweeps
over the whole sequence: H <- tanh(dv + W[2] (x) shift_t(H)), contracting by
~sum|W[2,:]| per sweep.

v2 layout (slot-major, fp16): SBUF partitions = (img(2) x j(64)); S[l] is
[128, 65 slots, 257 groups]: slot 0 group g = layer_input_g[row 63], slot
1+r group g = h_{g-1}[row r].  All DMAs are then contiguous runs of 256
fp16 along the group dim (512B descriptors).  Matmuls run in fp16 (1
PE cycle/row vs 4 for fp32), accumulating in fp32 PSUM.  Output rows
0..31 / 32..62 / 63 are DMA'd out incrementally so only the tiny row-63
transfer sits after the last compute.  The layer-2 bulk pass (rows 0..30,
which don't tap layer-1 row 63) is interleaved into iterate(0)'s serial
matmul->act latency bubbles.  The per-sweep dv constant is preloaded into
PSUM by the vector engine so each sweep is one matmul + one activation.
"""

import os
import numpy as np

B, L, D, NCORES = 16, 256, 64, 8
BS = B // NCORES          # images per core


def _band(w3):
    """[64,64] banded matrix M[jin, jout] = w3[jin-jout+1] for |jin-jout|<=1."""
    M = np.zeros((D, D), np.float32)
    for dj in range(3):
        jout = np.arange(D)
        jin = jout + dj - 1
        m = (jin >= 0) & (jin < D)
        M[jin[m], jout[m]] = w3[dj]
    return M


def _bands_tensor(Wn):
    """[128, 7, 128] fp16: (l, di) -> block-diag band; slot 6 = identity."""
    out = np.zeros((128, 7, 128), np.float32)
    for l in range(2):
        for di in range(3):
            M = _band(Wn[l, di])
            out[0:64, l * 3 + di, 0:64] = M
            out[64:128, l * 3 + di, 64:128] = M
    out[:, 6, :] = np.eye(128, dtype=np.float32)
    return np.ascontiguousarray(out.astype(np.float16))


def _conv1d3(v, w3):
    out = (w3[1] * v).copy()
    out[..., :-1] += w3[2] * v[..., 1:]
    out[..., 1:] += w3[0] * v[..., :-1]
    return out


def _numpy_layer(xl, Wl, bl, n_iter):
    """Reference decomposition (for sweep-count estimation). xl: (b,L,D,D)."""
    nb = xl.shape[0]
    h = np.zeros((nb, L, D, D), np.float32)
    xpad = np.zeros((nb, L, D + 2, D), np.float32)
    xpad[:, :, 1:D + 1] = xl
    for i in range(32):
        acc = np.zeros((nb, L, D), np.float32)
        for di in range(3):
            acc = acc + _conv1d3(xpad[:, :, 2 * i + di], Wl[di])
        h[:, :, i] = np.tanh(acc + bl)

    def S_prev(slot):
        out = np.zeros((nb, L, D), np.float32)
        if slot == 0:
            out[:, :] = xl[:, :, 63]
        else:
            out[:, 1:] = h[:, :-1, slot - 1]
        return out

    for lo, hi in ((32, 47), (48, 55), (56, 59), (60, 61), (62, 62)):
        for i in range(lo, hi + 1):
            acc = np.zeros((nb, L, D), np.float32)
            for di in range(3):
                acc = acc + _conv1d3(S_prev(2 * i - 64 + di), Wl[di])
            h[:, :, i] = np.tanh(acc + bl)

    dv = bl + _conv1d3(S_prev(62), Wl[0]) + _conv1d3(S_prev(63), Wl[1])
    H = np.zeros((nb, L, D), np.float32)
    deltas = []
    for _ in range(n_iter):
        Hp = np.zeros((nb, L, D), np.float32)
        Hp[:, 1:] = H[:, :-1]
        Hn = np.tanh(dv + _conv1d3(Hp, Wl[2]))
        deltas.append(float(np.abs(Hn - H).max()))
        H = Hn
    h[:, :, 63] = H
    return h, deltas


def _estimate_sweeps(x, Wn, bn, tol=1e-3):
    """Run the decomposition on one image, count sweeps until delta < tol.

    Harness gate is 2e-2 relative; fp16 storage floors the useful delta at
    ~2e-4, so iterating further is pure latency."""
    xs = x[:1].astype(np.float32)
    nits = []
    for l in range(2):
        xs_out, deltas = _numpy_layer(xs, Wn[l], bn[l], 30)
        nit = 30
        for k, d in enumerate(deltas):
            if d < tol:
                nit = k + 1
                break
        nits.append(min(30, max(5, nit + 2)))
        xs = xs_out
    return nits


def _build_bass(bn, nits):
    import concourse.bass as bass  # noqa: F401
    import concourse.bacc as bacc
    import concourse.mybir as mybir
    import concourse.tile as tile

    f16 = mybir.dt.float16
    f32 = mybir.dt.float32
    Tanh = mybir.ActivationFunctionType.Tanh

    nc = bacc.Bacc("TRN2", target_bir_lowering=False)
    # [part, t-quarter, row, t-in-quarter]: each quarter-DMA has a contiguous
    # source, quarters alternate between the two hwdge queues (sync/scalar)
    # so the first bulk chunk can start after ~1/4 of the input has landed.
    xT = nc.dram_tensor("xT", [128, 4, D, 64], f16, kind="ExternalInput")
    bands = nc.dram_tensor("bands", [128, 7, 128], f16, kind="ExternalInput")
    outT = nc.dram_tensor("outT", [128, D, L], f16, kind="ExternalOutput")

    with tile.TileContext(nc) as tc:
        with (
            tc.tile_pool(name="persist", bufs=1) as persist,
            tc.tile_pool(name="apool", bufs=3, space="PSUM") as apool,
            tc.tile_pool(name="ipool", bufs=2, space="PSUM") as ipool,
        ):
            bsb = persist.tile([128, 7, 128], f16)
            nc.sync.dma_start(out=bsb, in_=bands[:])

            def BD(l, di):
                return bsb[:, l * 3 + di, :]

            IDN = bsb[:, 6, :]

            S = [persist.tile([128, 65, L + 1], f16, name=f"S{i}")
                 for i in range(2)]
            dvs = [persist.tile([128, L], f16, name=f"dv{i}")
                   for i in range(2)]
            bias_t = [persist.tile([128, 1], f32, name=f"bias{i}")
                      for i in range(2)]
            for i in range(2):
                nc.vector.memset(bias_t[i][:, :], float(bn[i]))
                nc.vector.memset(S[i][:, 1:65, 0:1], 0.0)
                # slot 64 is read across all groups by the first fixed-point
                # sweep (H^0 = 0); stale SBUF there would poison row 63.
                nc.vector.memset(S[i][:, 64, :], 0.0)

            # t-quarter-major so each quarter's DMA is contiguous on both
            # sides (8KB/partition runs -> full HBM bandwidth).
            xt = persist.tile([128, 4, D, 64], f16)
            for c in range(4):
                nc.sync.dma_start(out=xt[:, c], in_=xT[:, c])
                # slot0[g] = x_g[row 63]
                nc.vector.tensor_copy(
                    S[0][:, 0, c * 64:(c + 1) * 64],
                    xt[:, c, 63, :])

            # Spin the PE on dummy matmuls while the input DMA streams in:
            # the PE pstate ramps with continuous execution, so the first
            # real chunks would otherwise run ~1.6x slow.
            for _ in range(16):
                pw = ipool.tile([128, 256], f32, name="pw", tag="it")
                nc.tensor.matmul(pw, bsb[:, 0, :], bsb[:, 0:2, :],
                                 start=True, stop=True)

            def bulk_chunk(l, ts):
                """Rows 0..31 (l=0) / 0..30 (l=1) for 32 timesteps at ts.

                PSUM is [128, 2, 32, 16]: each 16-t half fills one aligned
                2KB bank (matmuls may not cross banks), while one activation
                drains both banks through a 4D split-group output AP."""
                nrows = 32 if l == 0 else 31
                pa = apool.tile([128, 2, 32, 16], f32, name="pa", tag="acc")
                for q in range(2):
                    tq = ts + q * 16
                    if l == 0:
                        xq, lt = xt[:, tq // 64], tq % 64
                        r1 = xq[:, 0:63:2, lt:lt + 16]
                        r2 = xq[:, 1:64:2, lt:lt + 16]
                        r0 = xq[:, 1:62:2, lt:lt + 16]
                    else:
                        g = tq + 1
                        r1 = S[0][:, 1:62:2, g:g + 16]
                        r2 = S[0][:, 2:63:2, g:g + 16]
                        r0 = S[0][:, 2:61:2, g:g + 16]
                    nc.tensor.matmul(pa[:, q, 0:nrows, :], BD(l, 1), r1,
                                     start=True, stop=False)
                    nc.tensor.matmul(pa[:, q, 0:nrows, :], BD(l, 2), r2,
                                     start=False, stop=False)
                    nc.tensor.matmul(pa[:, q, 1:nrows, :], BD(l, 0), r0,
                                     start=False, stop=True)
                out = S[l][:, 1:1 + nrows, ts + 1:ts + 33].rearrange(
                    "p r (q t) -> p q r t", q=2)
                nc.scalar.activation(out, pa[:, :, 0:nrows, :], Tanh,
                                     bias=bias_t[l][:, :])

            def region_pass(l):
                """Cascade rows 32..62, region-major over the full sequence.
                Adjacent NT-tiles are paired into a 2-bank PSUM tile so one
                activation drains both."""
                Sl = S[l]
                for (ilo, ihi, NT) in ((32, 47, 32), (48, 55, 64),
                                       (56, 59, 128)):
                    n = ihi - ilo + 1
                    for t0 in range(0, L, 2 * NT):
                        pr = apool.tile([128, 2, n, NT], f32, name="pr",
                                        tag="acc")
                        for q in range(2):
                            tq = t0 + q * NT
                            for di in range(3):
                                s0 = 2 * ilo - 64 + di
                                rhs = Sl[:, s0:s0 + 2 * n - 1:2, tq:tq + NT]
                                nc.tensor.matmul(pr[:, q], BD(l, di), rhs,
                                                 start=(di == 0),
                                                 stop=(di == 2))
                        out = Sl[:, 1 + ilo:2 + ihi,
                                 t0 + 1:t0 + 2 * NT + 1].rearrange(
                                     "p r (q t) -> p q r t", q=2)
                        nc.scalar.activation(out, pr[:, :, :, :], Tanh,
                                             bias=bias_t[l][:, :])
                for (ilo, ihi) in ((60, 61), (62, 62)):
                    n = ihi - ilo + 1
                    pr = apool.tile([128, n, L], f32, name="prl", tag="acc")
                    for di in range(3):
                        s0 = 2 * ilo - 64 + di
                        rhs = Sl[:, s0:s0 + 2 * n - 1:2, 0:L]
                        nc.tensor.matmul(pr[:, :, :], BD(l, di), rhs,
                                         start=(di == 0), stop=(di == 2))
                    nc.scalar.activation(
                        Sl[:, 1 + ilo:2 + ihi, 1:L + 1],
                        pr[:, :, :], Tanh, bias=bias_t[l][:, :])

            def dv_prep(l):
                pd = ipool.tile([128, L], f32, name="pd", tag="it")
                nc.tensor.matmul(pd, BD(l, 0), S[l][:, 62, 0:L],
                                 start=True, stop=False)
                nc.tensor.matmul(pd, BD(l, 1), S[l][:, 63, 0:L],
                                 start=False, stop=True)
                nc.vector.tensor_copy(dvs[l][:, :], pd)

            def sweep(l):
                pi = ipool.tile([128, L], f32, name="pi", tag="it")
                nc.vector.tensor_copy(pi, dvs[l][:, :])
                nc.tensor.matmul(pi, BD(l, 2), S[l][:, 64, 0:L],
                                 start=False, stop=True,
                                 skip_group_check=True)
                nc.scalar.activation(S[l][:, 64, 1:L + 1], pi, Tanh,
                                     bias=bias_t[l][:, :])

            # ---- layer 1 ----
            for ts in range(0, L, 32):
                bulk_chunk(0, ts)
            region_pass(0)
            dv_prep(0)
            # iterate(0), with layer-2 bulk chunks filling the PE bubbles
            nchunks = L // 32
            for k in range(max(nits[0], nchunks)):
                if k < nits[0]:
                    sweep(0)
                if k < nchunks:
                    bulk_chunk(1, k * 32)
            # layer-2 row 31 (taps layer-1 rows 61,62,63 = slots 62,63,64)
            pr31 = apool.tile([128, L], f32, name="pr31", tag="acc")
            for di in range(3):
                nc.tensor.matmul(pr31, BD(1, di), S[0][:, 62 + di, 1:L + 1],
                                 start=(di == 0), stop=(di == 2))
            nc.scalar.activation(S[1][:, 32, 1:L + 1], pr31, Tanh,
                                 bias=bias_t[1][:, :])
            # layer-2 slot0[g] = h1_g[row 63]
            nc.vector.tensor_copy(S[1][:, 0, 0:L], S[0][:, 64, 1:L + 1])

            # rows 0..31 final -> overlap region_pass(1) with their DMA
            nc.sync.dma_start(out=outT[:, 0:32, :],
                              in_=S[1][:, 1:33, 1:L + 1])
            region_pass(1)
            # rows 32..62 final -> overlap iterate(1) with their DMA
            nc.sync.dma_start(out=outT[:, 32:63, :],
                              in_=S[1][:, 33:64, 1:L + 1])
            dv_prep(1)
            for k in range(nits[1]):
                sweep(1)
            nc.sync.dma_start(out=outT[:, 63, :], in_=S[1][:, 64, 1:L + 1])

    nc.compile()
    return nc


def kernel(x, W, b):
    import sys
    if "/opt/trn_rl_repo" not in sys.path:
        sys.path.insert(0, "/opt/trn_rl_repo")
    from concourse.bass_utils import run_bass_kernel_spmd

    x = np.ascontiguousarray(np.asarray(x, np.float32))
    Wn = np.asarray(W, np.float32)[:, 0, 0]      # (2, 3, 3)
    bn = np.asarray(b, np.float32)               # (2,)

    nits = _estimate_sweeps(x, Wn, bn)
    nc = _build_bass(bn, nits)

    bands_np = _bands_tensor(Wn)
    in_maps = []
    for c in range(NCORES):
        xc = x[c * BS:(c + 1) * BS]                      # (2, L, D, D)
        # (img, t, row, j) -> (img*j, row, t) -> [128, 2, 64, 128]
        xTc = xc.transpose(0, 3, 2, 1).reshape(128, D, L)
        xTc = np.ascontiguousarray(
            xTc.reshape(128, D, 4, 64).transpose(0, 2, 1, 3)
        ).astype(np.float16)
        in_maps.append({"xT": xTc, "bands": bands_np})

    res = run_bass_kernel_spmd(
        nc, in_maps, core_ids=list(range(NCORES)),
        trace=bool(int(os.environ.get("BASS_KERNEL_TRACE", "0"))))
    if os.environ.get("BASS_KERNEL_RESULT_PATH"):
        import pickle
        with open(os.environ["BASS_KERNEL_RESULT_PATH"], "wb") as f:
            pickle.dump({
                "exec_time_ns": res.exec_time_ns,
                "mean_exec_time_ns": res.mean_exec_time_ns,
                "trace": (res.instructions_and_trace or (None, None))[1],
                "profile_json": res.profile_json,
            }, f)

    out = np.empty((B, L, D, D), np.float32)
    for c in range(NCORES):
        r = res.results[c]
        main = r["outT"].reshape(BS, D, D, L)            # (img, j, row, t)
        out[c * BS:(c + 1) * BS] = main.transpose(0, 3, 2, 1).astype(np.float32)
    return out
